# revision 3
# baseline (speedup 1.0000x reference)
# Trainium2 Bass kernel for nn_CustomAttention (fused qkv + LoRA + per-head
# LayerNorm + softmax attention + output projection).
#
# Sharding: 16 heads over 8 cores (2 heads/core), both batches on every
# core; each core computes its heads' attention and its partial output
# projection, the host sums the 8 bf16 partials and adds proj_b. LoRA is
# folded into the qkv weights on the host.
#
# v5 (HW-profiled rework of the v2 schedule). Measured on-device engine
# stream costs showed the v2 kernel was gated by per-instruction costs the
# simulator underestimates: gpsimd ops ~950ns each (192 of them in the LN
# normalize/copy path, ~180us serial on the phase-A critical path), ACT
# activations ~(N+350)/1.2ns + ~520ns fixed, matmuls ~260ns per 512-col
# stream. Changes vs v2:
#  - LN normalize is 3 big DVE tensor_tensor ops per half (mean/rstd
#    broadcast along D) instead of 128 small gpsimd tensor_scalar ops;
#    the qkv weight layout is reordered to q0,k0,q1,k1,v0,v1 so q/k are
#    one contiguous block (and the two v's are adjacent for vp copies).
#  - Transposes packed: both heads' q (or k) in one 128-col stationary,
#    64 transposes instead of 128.
#  - sqs squaring on Pool (sbuf-only), stage/qkT/proj evacuations on DVE:
#    measured balance PE ~220us / ACT ~180 / DVE ~160 / Pool ~80.
#  - exp stream: es pool deepened to 8 buffers and exps emitted at
#    top scheduler priority; score matmuls too (they feed ACT).
#  - 4-deep cross-round score lookahead so the exp stream never waits
#    behind the projection tail; phase-A filler deprioritized so it only
#    fills real idle (avoids DVE head-of-line blocking).
#  - x DMA column-chunked so the first qkv tile starts ~3x earlier.
import numpy as np
import ml_dtypes

import concourse.bass as bass
import concourse.bacc as bacc
import concourse.mybir as mybir
from concourse.tile import TileContext
from concourse.masks import make_identity
from concourse.bass_utils import run_bass_kernel_spmd

BF16 = ml_dtypes.bfloat16
F32 = np.float32

B, N, DIM, H, R = 2, 2048, 1024, 16, 8
D = DIM // H              # 64
NCORES = 8
HPC = H // NCORES         # 2 heads per core
ALPHA = 8.0
LORA_SCALE = ALPHA / R
EPS = 1e-5
QSCALE = float(D) ** -0.5  # 0.125

NCH = DIM // 128          # 8 contraction chunks of 128
NTI = N // 128            # 16 row tiles of 128
QI = 512                  # query-block width (free dim of score matmuls)
NIB = N // QI             # 4 query blocks per batch
HT = NTI // 2             # 8 row tiles per LN-stats half

_prog_cache: dict = {}


def _build_program(use_mask: bool, affine_q: bool, affine_k: bool, repeat: int = 1):
    nc = bacc.Bacc("TRN2", target_bir_lowering=False)
    f32 = mybir.dt.float32
    bf16 = mybir.dt.bfloat16

    xT = nc.dram_tensor("xT", [128, B, NCH, N], bf16, kind="ExternalInput")
    wT = nc.dram_tensor("wT", [NCH, 128, 6 * D], bf16, kind="ExternalInput")
    projT = nc.dram_tensor("projT", [128, DIM], bf16, kind="ExternalInput")
    out_p = nc.dram_tensor("out_p", [128, B, NTI, DIM], bf16, kind="ExternalOutput")
    if affine_q or affine_k:
        lnaff = nc.dram_tensor("lnaff", [4, 128, D], f32, kind="ExternalInput")
    if use_mask:
        emaskT = nc.dram_tensor("emaskT", [N, N], bf16, kind="ExternalInput")

    with TileContext(nc) as tc:
        import contextlib
        with contextlib.ExitStack() as ctx:
            const = ctx.enter_context(tc.tile_pool(name="const", bufs=1))
            ident = const.tile([128, 128], bf16)
            make_identity(nc, ident)
            eps_t = const.tile([128, 1], f32)
            nc.vector.memset(eps_t, EPS)
            from concourse.hw_specs import get_activation_tables
            try:
                _tnames = list(get_activation_tables(nc.m.arch).keys())
                _set_id = _tnames.index("natural_log_exp_and_others")
            except Exception:
                _set_id = 6
            nc.scalar.add_instruction(
                mybir.InstLoadActFuncSet(
                    name=nc.get_next_instruction_name(), ins=[], outs=[],
                    act_func_set_id=_set_id))

            persist = ctx.enter_context(tc.tile_pool(name="persist", bufs=1))
            w_sb = persist.tile([128, NCH, 6 * D], bf16)
            nc.sync.dma_start(out=w_sb, in_=wT.rearrange("ci cm w -> cm ci w"))
            proj_sb = persist.tile([128, DIM], bf16)
            nc.sync.dma_start(out=proj_sb, in_=projT[:, :])
            if affine_q or affine_k:
                aff_sb = persist.tile([128, 4, D], f32)
                nc.sync.dma_start(out=aff_sb, in_=lnaff.rearrange("r p d -> p r d"))
            vps = []
            for par in range(B):
                vp = persist.tile([128, NTI, HPC, 128], bf16, name=f"vp{par}")
                nc.vector.memset(vp[:, :, 0, D:], 1.0)
                nc.vector.memset(vp[:, :, 1, :D], 1.0)
                vps.append(vp)

            xpool = ctx.enter_context(tc.tile_pool(name="xpool", bufs=2))
            qkpool = ctx.enter_context(tc.tile_pool(name="qkpool", bufs=2))
            stg = ctx.enter_context(tc.tile_pool(name="stg", bufs=2))
            sqp = ctx.enter_context(tc.tile_pool(name="sqp", bufs=2))
            lnp = ctx.enter_context(tc.tile_pool(name="lnp", bufs=2))
            natp = ctx.enter_context(tc.tile_pool(name="natp", bufs=8))
            nlp = ctx.enter_context(tc.tile_pool(name="nlp", bufs=2))
            esp = ctx.enter_context(tc.tile_pool(name="esp", bufs=8))
            otp = ctx.enter_context(tc.tile_pool(name="otp", bufs=4))
            outp = ctx.enter_context(tc.tile_pool(name="outp", bufs=1))
            if use_mask:
                mskp = ctx.enter_context(tc.tile_pool(name="mskp", bufs=4))
            psS = ctx.enter_context(tc.tile_pool(name="psS", bufs=2, space="PSUM"))
            psB = ctx.enter_context(tc.tile_pool(name="psB", bufs=2, space="PSUM"))
            psA = ctx.enter_context(tc.tile_pool(name="psA", bufs=2, space="PSUM"))

            if repeat > 1:
                ctx.enter_context(tc.For_i(
                    0, repeat, 1,
                    hint_engines=(mybir.EngineType.PE, mybir.EngineType.SP,
                                  mybir.EngineType.Activation,
                                  mybir.EngineType.DVE, mybir.EngineType.Pool)))

            # ---- hoisted input loads for both batches ----
            # column-chunked so qkv ti0 starts after ~1MB, not the full 4.2MB
            x_sbs = []
            for b in range(B):
                x_sb = xpool.tile([128, NCH, N], bf16, tag="x_sb")
                for nq in range(4):
                    nsl = slice(nq * (N // 4), (nq + 1) * (N // 4))
                    nc.sync.dma_start(out=x_sb[:, :, nsl],
                                      in_=xT[:, b, :, nsl])
                x_sbs.append(x_sb)

            # ---------------- phase A emission (fine-grained steps) --------
            def make_A_steps(b, borrow_sT=False, fine=False):
                hd = {}
                x_sb = x_sbs[b]

                def alloc():
                    qkT = qkpool.tile([128, 2, N], bf16, tag="qkT", name="qkT")
                    hd["qT"] = qkT[:, 0, :]
                    hd["kT"] = qkT[:, 1, :]
                    hd["qkT"] = qkT
                    hd["stage"] = stg.tile([128, NTI, 6 * D], bf16, tag="stage", name="stage")
                    hd["sqs"] = sqp.tile([128, NTI, 6 * D], bf16, tag="sqs", name="sqs")
                    # normalized q/k, laid out [ti, qk, hh, D] so each
                    # packed transpose reads one contiguous 128-col block
                    hd["natall"] = nlp.tile([128, NTI, 2, 2, D], bf16,
                                            tag="natall", name="natall")

                def qkv_ti(ti, half=None):
                    # half=None: all 8 chunks; 0/1: chunks 0-3 / 4-7
                    def f():
                        stage = hd["stage"]; sqs = hd["sqs"]
                        if borrow_sT:
                            if half in (None, 0):
                                hd["pqt"] = psS.tile([128, 2, QI], f32,
                                                     tag="sT", name="pqt")
                            pq = hd["pqt"][:, 0, :]
                        else:
                            if half in (None, 0):
                                hd["pq"] = psA.tile([128, 512], f32,
                                                    tag="pq", bufs=1, name="pq")
                            pq = hd["pq"]
                        cis = range(NCH) if half is None else range(half * 4, half * 4 + 4)
                        for ci in cis:
                            nc.tensor.matmul(
                                pq[:, 0:6 * D],
                                lhsT=x_sb[:, ci, ti * 128:(ti + 1) * 128],
                                rhs=w_sb[:, ci, :],
                                start=(ci == 0), stop=(ci == NCH - 1),
                            )
                        if half in (None, 1):
                            nc.vector.tensor_copy(out=stage[:, ti, :], in_=pq[:, 0:6 * D])
                            # squares on the otherwise-idle Pool engine
                            nc.gpsimd.tensor_tensor(
                                out=sqs[:, ti, :], in0=stage[:, ti, :],
                                in1=stage[:, ti, :], op=mybir.AluOpType.mult)
                    return f

                def stats_dve(half):
                    # DVE-only part: reduces + scalar ops producing var
                    def f():
                        stage = hd["stage"]; sqs = hd["sqs"]
                        st6v = stage.rearrange("p t (i d) -> p t i d", d=D)
                        sq6v = sqs.rearrange("p t (i d) -> p t i d", d=D)
                        hsl = slice(half * HT, (half + 1) * HT)
                        mean = lnp.tile([128, HT, 6], bf16, tag="meanh")
                        with nc.allow_low_precision(reason="LN stats: bf16 out, fp32 internal accum"):
                            nc.vector.tensor_reduce(
                                out=mean, in_=st6v[:, hsl], axis=mybir.AxisListType.X,
                                op=mybir.AluOpType.add)
                        meanf = lnp.tile([128, HT, 6], f32, tag="mean")
                        nc.vector.tensor_scalar(
                            out=meanf, in0=mean, scalar1=1.0 / D, scalar2=None,
                            op0=mybir.AluOpType.mult)
                        varh = lnp.tile([128, HT, 6], bf16, tag="varh")
                        with nc.allow_low_precision(reason="LN stats: bf16 out, fp32 internal accum"):
                            nc.vector.tensor_reduce(
                                out=varh, in_=sq6v[:, hsl], axis=mybir.AxisListType.X,
                                op=mybir.AluOpType.add)
                        var = lnp.tile([128, HT, 6], f32, tag="var")
                        nc.vector.tensor_scalar(
                            out=var, in0=varh, scalar1=1.0 / D, scalar2=None,
                            op0=mybir.AluOpType.mult)
                        m2 = lnp.tile([128, HT, 6], f32, tag="m2")
                        nc.vector.tensor_tensor(
                            out=m2, in0=meanf, in1=meanf, op=mybir.AluOpType.mult)
                        nc.vector.tensor_tensor(
                            out=var, in0=var, in1=m2, op=mybir.AluOpType.subtract)
                        hd[("mean", half)] = meanf
                        hd[("var", half)] = var
                    return f

                def stats_act(half):
                    # ACT part (Ln+Exp): pulled well after stats_dve so the
                    # in-order ACT queue never blocks the exp stream on a
                    # not-yet-ready var
                    def f():
                        var = hd[("var", half)]
                        lnv = lnp.tile([128, HT, 6], f32, tag="lnv")
                        nc.scalar.activation(
                            out=lnv, in_=var,
                            func=mybir.ActivationFunctionType.Ln,
                            bias=eps_t, scale=1.0)
                        rstd = lnp.tile([128, HT, 6], f32, tag="rstd")
                        nc.scalar.activation(
                            out=rstd, in_=lnv,
                            func=mybir.ActivationFunctionType.Exp,
                            scale=-0.5)
                        if not affine_q:
                            # q insts are 0 and 2 (layout q0,k0,q1,k1,v0,v1)
                            nc.vector.tensor_scalar(
                                out=rstd[:, :, 0:3:2], in0=rstd[:, :, 0:3:2],
                                scalar1=QSCALE, scalar2=None,
                                op0=mybir.AluOpType.mult)
                        hd[("rstd", half)] = rstd
                    return f

                def nat_half(half):
                    # LN-normalize q/k for a whole half in two big DVE ops
                    # (mean/rstd broadcast along D) — replaces 64 small
                    # gpsimd tensor_scalar ops (~950ns each on HW)
                    def f():
                        stage = hd["stage"]
                        st6v = stage.rearrange("p t (i d) -> p t i d", d=D)
                        natall = hd["natall"]
                        mean = hd[("mean", half)]; rstd = hd[("rstd", half)]
                        hsl = slice(half * HT, (half + 1) * HT)
                        mb = mean[:, :, 0:4].unsqueeze(3) \
                            .broadcast_to([128, HT, 4, D])
                        rb = rstd[:, :, 0:4].unsqueeze(3) \
                            .broadcast_to([128, HT, 4, D])
                        # sqs half is dead after the var reduce — reuse as tmp
                        tmp = hd["sqs"].rearrange(
                            "p t (i d) -> p t i d", d=D)[:, hsl, 0:4, :]
                        nc.vector.tensor_tensor(
                            out=tmp, in0=st6v[:, hsl, 0:4, :], in1=mb,
                            op=mybir.AluOpType.subtract)
                        # ISA free-dim APs are max 3D: one mult per qk
                        tmp4 = tmp.rearrange("p t (h a) d -> p t h a d", a=2)
                        for qk in range(2):
                            rbq = rstd[:, :, qk:4:2].unsqueeze(3) \
                                .broadcast_to([128, HT, 2, D])
                            nc.vector.tensor_tensor(
                                out=natall[:, hsl, qk], in0=tmp4[:, :, :, qk],
                                in1=rbq, op=mybir.AluOpType.mult)
                    return f

                def natT_ti(half, tih):
                    def f():
                        stage = hd["stage"]
                        st6v = stage.rearrange("p t (i d) -> p t i d", d=D)
                        mean = hd[("mean", half)]; rstd = hd[("rstd", half)]
                        natall = hd["natall"]
                        ti = half * HT + tih
                        pt = psA.tile([128, 2, 512], bf16, tag="pt", bufs=1)
                        if affine_q or affine_k:
                            # rare path: per-inst gpsimd normalize + affine
                            for inst, qk, hh in ((0, 0, 0), (2, 0, 1),
                                                 (1, 1, 0), (3, 1, 1)):
                                affine = affine_q if qk == 0 else affine_k
                                nat = natp.tile([128, D], bf16, tag="nat")
                                if affine:
                                    natf = natp.tile([128, D], f32, tag="natf")
                                    nc.gpsimd.tensor_scalar(
                                        out=natf, in0=st6v[:, ti, inst, :],
                                        scalar1=mean[:, tih, inst:inst + 1],
                                        scalar2=rstd[:, tih, inst:inst + 1],
                                        op0=mybir.AluOpType.subtract,
                                        op1=mybir.AluOpType.mult)
                                    r = 0 if qk == 0 else 2
                                    natf2 = natp.tile([128, D], f32, tag="natf2")
                                    nc.gpsimd.tensor_tensor(
                                        out=natf2, in0=natf, in1=aff_sb[:, r, :],
                                        op=mybir.AluOpType.mult)
                                    nc.gpsimd.tensor_tensor(
                                        out=nat, in0=natf2, in1=aff_sb[:, r + 1, :],
                                        op=mybir.AluOpType.add)
                                else:
                                    nc.gpsimd.tensor_scalar(
                                        out=nat, in0=st6v[:, ti, inst, :],
                                        scalar1=mean[:, tih, inst:inst + 1],
                                        scalar2=rstd[:, tih, inst:inst + 1],
                                        op0=mybir.AluOpType.subtract,
                                        op1=mybir.AluOpType.mult)
                                nc.tensor.transpose(
                                    pt[hh * D:(hh + 1) * D, qk, 0:128], nat, ident)
                        else:
                            # packed transposes: both heads' q (insts 0,2) in
                            # one 128-col stationary, likewise k (insts 1,3)
                            for qk in range(2):
                                nc.tensor.transpose(
                                    pt[:, qk, 0:128],
                                    natall[:, ti, qk], ident)
                        nc.vector.tensor_copy(
                            out=hd["qkT"][:, :, ti * 128:(ti + 1) * 128],
                            in_=pt[:, :, 0:128])
                        nc.gpsimd.tensor_copy(
                            out=vps[b][:, ti, 0, 0:D], in_=st6v[:, ti, 4, :])
                        nc.gpsimd.tensor_copy(
                            out=vps[b][:, ti, 1, D:], in_=st6v[:, ti, 5, :])
                    return f

                def emit_qkv(half):
                    out = []
                    for tih in range(HT):
                        ti = half * HT + tih
                        if fine:
                            out.append(qkv_ti(ti, 0))
                            out.append(qkv_ti(ti, 1))
                        else:
                            out.append(qkv_ti(ti))
                    return out

                # ordering: each stats_act is separated from its stats_dve by
                # a stretch of other work so the ACT queue never stalls on it
                steps = [alloc]
                steps += emit_qkv(0)
                steps.append(stats_dve(0))
                steps += emit_qkv(1)
                steps.append(stats_act(0))
                steps.append(nat_half(0))
                steps.append(stats_dve(1))
                for tih in range(HT):
                    steps.append(natT_ti(0, tih))
                steps.append(stats_act(1))
                steps.append(nat_half(1))
                for tih in range(HT):
                    steps.append(natT_ti(1, tih))
                return steps, hd

            def run_all(steps):
                for s in steps:
                    s()

            # ---------------- attention rounds ------------------------------
            hds = {}

            def emit_scores(b, ib, jp):
                qT = hds[b]["qT"]; kT = hds[b]["kT"]
                i0 = ib * QI
                sTs = []
                # top priority band: scores feed the exp stream (ACT is the
                # pacing engine) — they must win the PE queue the moment
                # their PSUM bank frees, never queue behind phase-A work
                with tc.high_priority():
                    for hh in range(HPC):
                        sTs.append(psS.tile([128, 2, QI], f32, tag="sT", name="sT"))
                    for cj in range(2):
                        j = jp * 2 + cj
                        for hh in range(HPC):
                            hs = slice(hh * D, (hh + 1) * D)
                            nc.tensor.matmul(
                                sTs[hh][:, cj, :],
                                lhsT=kT[hs, j * 128:(j + 1) * 128],
                                rhs=qT[hs, i0:i0 + QI],
                                start=True, stop=True,
                            )
                return sTs

            def emit_round(b, ib, filler, pre2, nxt, ppj=1):
                """One (b, ib) attention round. pre2: [sTs_jp0, sTs_jp1]
                pre-emitted by the previous round's lookahead (or None).
                nxt: next round (b', ib') or None — its jp0 scores are
                emitted at this round's jp7, its jp1 scores after av jp7
                (both ahead of the proj matmuls in the PE queue), so the
                exp stream never waits behind the projection tail."""
                def pull(n=1):
                    for _ in range(n):
                        s = next(filler, None)
                        if s is not None:
                            # deprioritized: phase-A filler only fills real
                            # idle, never outranks round work in any engine
                            # stream (avoids DVE head-of-line blocking)
                            with tc.high_priority(offset=-100000):
                                s()
                vp = vps[b]
                i0 = ib * QI
                avs = []
                for hh in range(HPC):
                    avs.append(psB.tile([128, QI], f32, tag="av", name="av"))
                if pre2 is not None:
                    sTs = pre2[0]
                    pending = list(pre2[1:])
                else:
                    sTs = emit_scores(b, ib, 0)
                    pending = []
                la = []
                for jp in range(8):
                    ess = []
                    for hh in range(HPC):
                        ess.append(esp.tile([128, 2, QI], bf16, tag="es", name="es"))
                    with tc.high_priority():
                        for hh in range(HPC):
                            nc.scalar.activation(
                                out=ess[hh], in_=sTs[hh],
                                func=mybir.ActivationFunctionType.Exp)
                    if use_mask:
                        for hh in range(HPC):
                            msk = mskp.tile([128, 2, QI], bf16, tag="msk")
                            for cj in range(2):
                                j = jp * 2 + cj
                                nc.sync.dma_start(
                                    out=msk[:, cj, :],
                                    in_=emaskT[j * 128:(j + 1) * 128,
                                               i0:i0 + QI])
                            nc.vector.tensor_tensor(
                                out=ess[hh], in0=ess[hh], in1=msk,
                                op=mybir.AluOpType.mult)
                    # queue next scores ahead of the exp-gated av matmuls
                    if pending:
                        upcoming = pending.pop(0)
                    elif jp + 1 < 8:
                        upcoming = emit_scores(b, ib, jp + 1)
                    elif nxt is not None:
                        la.append(emit_scores(nxt[0], nxt[1], 0))
                        upcoming = None
                    else:
                        upcoming = None
                    pull(ppj)
                    for hh in range(HPC):
                        for cj in range(2):
                            j = jp * 2 + cj
                            nc.tensor.matmul(
                                avs[hh],
                                lhsT=vp[:, j, hh, :],
                                rhs=ess[hh][:, cj, :],
                                start=(j == 0), stop=(j == NTI - 1),
                            )
                    sTs = upcoming
                # deep lookahead: next round's jp1-jp3 scores are emitted
                # ahead of / inside the projection tail so the exp stream
                # never waits behind the DVE-copy-gated proj block. The
                # allocations legally wait on the sT 2-buffer rotation.
                if nxt is not None:
                    la.append(emit_scores(nxt[0], nxt[1], 1))
                # normalization: av0 = [out|den], av1 = [den|out]
                zr = otp.tile([128, QI], f32, tag="zr")
                nc.vector.reciprocal(out=zr[0:D, :], in_=avs[0][D:, :])
                nc.vector.reciprocal(out=zr[D:, :], in_=avs[1][0:D, :])
                oT2 = otp.tile([128, QI], bf16, tag="oT2")
                nc.vector.tensor_tensor(
                    out=oT2[0:D, :], in0=avs[0][0:D, :], in1=zr[0:D, :],
                    op=mybir.AluOpType.mult)
                nc.vector.tensor_tensor(
                    out=oT2[D:, :], in0=avs[1][D:, :], in1=zr[D:, :],
                    op=mybir.AluOpType.mult)
                pull(1)
                osb = outp.tile([128, QI // 128, DIM], bf16, tag="osb")
                for sub in range(QI // 128):
                    for nh in range(2):
                        pp = psB.tile([128, 512], f32, tag="av", name="pp")
                        nc.tensor.matmul(
                            pp,
                            lhsT=oT2[:, sub * 128:(sub + 1) * 128],
                            rhs=proj_sb[:, nh * 512:(nh + 1) * 512],
                            start=True, stop=True,
                        )
                        nc.vector.tensor_copy(
                            out=osb[:, sub, nh * 512:(nh + 1) * 512], in_=pp)
                    if nxt is not None and sub in (0, 2):
                        la.append(emit_scores(nxt[0], nxt[1], 2 if sub == 0 else 3))
                    pull(1)
                ti0 = ib * (QI // 128)
                nc.sync.dma_start(
                    out=out_p[:, b, ti0:ti0 + QI // 128, :], in_=osb)
                return la if nxt is not None else None

            # ---------------- schedule --------------------------------------
            stepsA0, hd0 = make_A_steps(0, borrow_sT=True)
            run_all(stepsA0)
            hds[0] = hd0
            stepsA1, hd1 = make_A_steps(1, fine=True)
            hds[1] = hd1
            fillA1 = iter(stepsA1)
            rounds = [(0, ib) for ib in range(NIB)] + \
                     [(1, ib) for ib in range(NIB)]
            pre = None
            for ri, (b, ib) in enumerate(rounds):
                nxt = rounds[ri + 1] if ri + 1 < len(rounds) else None
                # front-load the A1 filler (phase-A critical chain must
                # finish early; measured faster than spreading it evenly)
                ppj = 3 if ri < 2 else 1
                pre = emit_round(b, ib, fillA1, pre, nxt, ppj=ppj)
                if (b, ib) == (0, NIB - 1):
                    for s in fillA1:
                        s()
                    fillA1 = iter(())
    nc.compile()
    return nc


def _prep_inputs(inputs):
    x = np.ascontiguousarray(inputs["x"], dtype=F32)
    qkv_w = np.asarray(inputs["qkv_w"], dtype=F32)
    proj_w = np.asarray(inputs["proj_w"], dtype=F32)
    W_eff = qkv_w.copy()
    for i, (a, bm) in enumerate([("lora_Aq", "lora_Bq"), ("lora_Ak", "lora_Bk"),
                                 ("lora_Av", "lora_Bv")]):
        A = np.asarray(inputs[a], dtype=F32)
        Bm = np.asarray(inputs[bm], dtype=F32)
        W_eff[i * DIM:(i + 1) * DIM] += LORA_SCALE * (A @ Bm).T

    xT_all = np.ascontiguousarray(
        x.transpose(2, 0, 1).reshape(NCH, 128, B, N)
        .transpose(1, 2, 0, 3).astype(BF16))

    qn_w = np.asarray(inputs["qn_w"], F32); qn_b = np.asarray(inputs["qn_b"], F32)
    kn_w = np.asarray(inputs["kn_w"], F32); kn_b = np.asarray(inputs["kn_b"], F32)
    affine_q = not (np.all(qn_w == 1.0) and np.all(qn_b == 0.0))
    affine_k = not (np.all(kn_w == 1.0) and np.all(kn_b == 0.0))
    mask = np.asarray(inputs["attn_mask"], F32)
    use_mask = bool(np.any(mask))

    common = {"xT": xT_all}
    if affine_q or affine_k:
        aff = np.stack([
            np.broadcast_to(qn_w * QSCALE, (128, D)),
            np.broadcast_to(qn_b * QSCALE, (128, D)),
            np.broadcast_to(kn_w, (128, D)),
            np.broadcast_to(kn_b, (128, D)),
        ]).astype(F32)
        common["lnaff"] = np.ascontiguousarray(aff)
    if use_mask:
        common["emaskT"] = np.ascontiguousarray(
            np.exp(mask[0, 0].T).astype(BF16))

    in_maps = []
    for c in range(NCORES):
        h0 = c * HPC
        blocks = []
        # layout q0,k0,q1,k1,v0,v1 so q/k are contiguous for the bulk
        # LN-normalize and v pair is adjacent for the vp copies
        for hh in range(HPC):
            h = h0 + hh
            for part in range(2):  # q, k
                blocks.append(W_eff[part * DIM + h * D: part * DIM + (h + 1) * D])
        for hh in range(HPC):
            h = h0 + hh
            blocks.append(W_eff[2 * DIM + h * D: 2 * DIM + (h + 1) * D])
        Wlocal = np.concatenate(blocks, axis=0)          # [384, 1024]
        wT_c = np.ascontiguousarray(
            Wlocal.T.reshape(NCH, 128, 6 * D).astype(BF16))
        projT_c = np.ascontiguousarray(np.concatenate(
            [proj_w[:, (h0 + hh) * D:(h0 + hh + 1) * D].T for hh in range(HPC)],
            axis=0).astype(BF16))                        # [128, 1024]
        m = dict(common)
        m["wT"] = wT_c
        m["projT"] = projT_c
        in_maps.append(m)
    return in_maps, (use_mask, affine_q, affine_k)


def _run(inputs, trace=False):
    in_maps, key = _prep_inputs(inputs)
    if key not in _prog_cache:
        _prog_cache[key] = _build_program(*key)
    nc = _prog_cache[key]
    res = run_bass_kernel_spmd(nc, in_maps, core_ids=list(range(NCORES)),
                               trace=trace)
    acc = np.zeros((128, B, NTI, DIM), dtype=F32)
    for r in res.results:
        acc += r["out_p"].astype(F32)
    out = np.ascontiguousarray(acc.transpose(1, 2, 0, 3).reshape(B, N, DIM))
    out += np.asarray(inputs["proj_b"], F32)
    return out, res


def kernel(**inputs) -> np.ndarray:
    out, _ = _run(inputs)
    return out
